# revision 65
# baseline (speedup 1.0000x reference)
"""Trainium2 Bass kernel for GQA attention (B=4, S=2048, HID=896, H=14, KV=2, D=64).

Sharding: 8 cores = 4 batches x 2 KV-head groups. Core c handles batch c//2,
query heads [g*7, (g+1)*7) with g = c%2 (exactly one KV head per core thanks to
GQA group structure). Each core computes its 448-channel slice of attn output
and the partial output projection y_g = ao_g @ Wo[g*448:(g+1)*448, :]; the host
sums the two partials per batch (f16 partials, f32 host sum).

Per-core pipeline (S=2048):
  A)  q = x@Wq_g (448 ch), kv = x@[Wk|Wv]_g (128 ch) from one fused weight
      tile, lhsT = x^T tiles (loaded in s-halves so compute starts early).
      RoPE on DVE in f16 at 2x rate: the per-head channels are host-permuted
      to [evens | odds] blocks so the even/odd views are contiguous (scores
      are invariant to a shared q/k row permutation). PE-transpose q and k to
      channel-major; k RoPE + transposes + kT/q half-duplication emitted in
      4-s-tile chunks inline with the loop (no serial tail). kT2 holds k in
      both partition halves; q_pair holds heads (2j, 2j+1) at halves (0, 64)
      and qx the half-swap, so every head's scores can issue from EITHER PE
      row-group half.
  B+C) One flat software-pipelined stream over all (qc, head, k-tile-pair)
      units, qc outer ascending. scoresT[k,q] = kT.T @ qT (K=64): the pair's
      two matmuls run CONCURRENTLY on the PE (row-groups 0-1 / 2-3 via base
      partitions 0 / 64, ~512 cycles per packed pair); one [128,1024] PSUM
      tile + one exp per pair keeps ACT ~95% saturated -- the exp stream is
      the kernel's critical path. Structural causality (only k-tiles <=
      diagonal; diagonal tiles masked with a triangular 0/1 multiply after
      exp). attn@[v|1] accumulates into PSUM [65, 512]; row 64 = softmax
      denominators. After a qc's 7 heads finish: 1/den via the custom-DVE
      fast reciprocal (no ACT tables, no Ln/Exp), DRAM-roundtrip partition
      broadcast, in-place normalize of the raw attn rows, then the qc's
      output projection y = ao @ Wo_g (split 512/384 columns, [128,512] PSUM)
      -> f16 y. The normalize multiplies and projection passes are AGE-GATED
      deferred work, drip-fed into the engine streams a few pipeline steps
      after their DMA roundtrips were issued, so the in-order DVE/PE streams
      never head-of-line block on a DMA (projections additionally wait until
      their qc's normalize multiplies are emitted -- correctness).

The causal mask input is never loaded: exp(-1e9 + s) == 0.0 exactly in fp32,
so structural masking matches the reference's additive mask bit-for-bit.
PSUM budget: scores 2x[128,1024] (4 banks) + attn@v 2x[65,512] (2) + output
projection 2x[128,512] (2) = 8 banks.

mm_dt selects the matmul dtype: float32r (2 PE cycles/row, ~1.4e-4 rounding =
RNE to 11 mantissa bits, replicated host-side) or float16 (1 cycle/row, ~2x
the rounding error) or float32 (exact, 4 cycles/row).
"""
import math
import os
import numpy as np

import concourse.bass as bass
import concourse.mybir as mybir
import concourse.tile as tile
from concourse import bacc
from concourse.masks import make_identity

F32 = mybir.dt.float32
F32R = mybir.dt.float32r
F16 = mybir.dt.float16
AF = mybir.ActivationFunctionType

B, S, HID = 4, 2048, 896
H, KV, D = 14, 2, 64
HL = H // KV          # 7 local query heads per core
GD = HL * D           # 448 local channels
KCH = HID // 128      # 7 contraction chunks
N_CORES = 8


def _bc7(ap_small):
    """[128, 32] cos/sin slice -> broadcast over the 7 heads: [128, 7, 32]."""
    return bass.AP(
        tensor=ap_small.tensor,
        offset=ap_small.offset,
        ap=[list(ap_small.ap[0]), [0, HL], list(ap_small.ap[1])],
    )


def build(s=S, mm_dt=None, reps=1, phases="A2BC"):
    if mm_dt is None:
        mm_dt = MM_DT
    ST = s // 128           # s-tiles
    QC = s // 512           # q chunks
    nc = bacc.Bacc("TRN2", target_bir_lowering=False, debug=False,
                   num_devices=N_CORES)

    xT = nc.dram_tensor("xT", [HID, s], mm_dt, kind="ExternalInput").ap()
    wq = nc.dram_tensor("wq", [HID, GD + 128], mm_dt, kind="ExternalInput").ap()
    wo = nc.dram_tensor("wo", [GD, HID], mm_dt, kind="ExternalInput").ap()
    cosr = nc.dram_tensor("cosr", [128, ST, 32], F32, kind="ExternalInput").ap()
    sinr = nc.dram_tensor("sinr", [128, ST, 32], F32, kind="ExternalInput").ap()
    triu = nc.dram_tensor("triu", [128, 128], mm_dt, kind="ExternalInput").ap()
    y = nc.dram_tensor("y", [s, HID], F16, kind="ExternalOutput").ap()
    rec_dt = F16
    dram_rec = nc.dram_tensor("dram_rec", [HL, s], rec_dt).ap()

    with tile.TileContext(nc) as tc:
        with (
            tc.tile_pool(name="wp", bufs=1) as wp,
            tc.tile_pool(name="per", bufs=1) as per,
            tc.tile_pool(name="tmp", bufs=2) as tmp,
        ):
            def _body():
                # ---- small loads (weights/tables stay resident) ----
                # DMA emission order = queue order: wq/wkv first (phase A
                # needs them immediately; xT follows in phase A's pool), then
                # tables, then wo LAST (only phase C reads it) so the
                # DMA-bound lead-in gates on ~4.4MB instead of ~5.7MB.
                # input loads issue from the ACT engine's HW-DGE queue: a
                # separate DMA FIFO from the sync queue that carries the
                # previous rep's tail stores, and ACT's instruction stream
                # frees at the previous rep's last exp -- so the next rep's
                # transfers start ~25us earlier in the reps loop
                wq_sb = wp.tile([128, KCH, GD + 128], mm_dt, tag="wq", name="wq")
                nc.scalar.dma_start(out=wq_sb[:], in_=wq.rearrange("(k p) m -> p k m", p=128))
                cos_sb = wp.tile([128, ST, 32], F32, tag="cos", name="cos")
                sin_sb = wp.tile([128, ST, 32], F32, tag="sin", name="sin")
                triu_sb = wp.tile([128, 128], mm_dt, tag="triu", name="triu")
                wo_sb = wp.tile([128, 4, HID], mm_dt, tag="wo", name="wo")

                def _late_loads():
                    nc.scalar.dma_start(out=cos_sb[:], in_=cosr)
                    nc.scalar.dma_start(out=sin_sb[:], in_=sinr)
                    nc.sync.dma_start(out=triu_sb[:], in_=triu)
                    for cc in range(4):
                        w = 128 if cc < 3 else 64
                        nc.sync.dma_start(out=wo_sb[0:w, cc, :],
                                          in_=wo[cc * 128:cc * 128 + w, :])
                # f16 copies of the RoPE tables for the 2x-rate q RoPE
                cos16 = wp.tile([128, ST, 32], F16, tag="cos16", name="cos16")
                sin16 = wp.tile([128, ST, 32], F16, tag="sin16", name="sin16")

                def _table_casts():
                    nc.vector.tensor_copy(cos16[:], cos_sb[:])
                    nc.vector.tensor_copy(sin16[:], sin_sb[:])

                # PE warmup (single-shot only: in the reps loop the HAM
                # clock-gate is already warm): dummy matmuls on a memset tile
                # start immediately and overlap the input-DMA lead-in
                wdum = wp.tile([128, 128], mm_dt, tag="wdum", name="wdum")
                nc.vector.memset(wdum[:], 0.0)
                with tc.tile_pool(name="psW", bufs=1, space="PSUM") as psW:
                    wps = psW.tile([128, 128], F32, tag="w", name="w")
                    for _ in range(72 if reps == 1 else 0):
                        nc.tensor.matmul(wps[:], wdum[:], wdum[:],
                                         start=True, stop=True)
                idn = wp.tile([128, 128], F32, tag="idn", name="idn")
                make_identity(nc, idn[:])
                idn_r = wp.tile([128, 128], mm_dt, tag="idnr", name="idnr")
                nc.vector.tensor_copy(idn_r[:], idn[:])
                idn_mm = idn_r[:]

                # ---- persistent intermediates ----
                # qT in head-pair chunks: q_pair[j] holds heads (2j, 2j+1) at
                # halves (0, 64); qx[j] is the half-swap of q_pair[j] so
                # every head has its q at BOTH halves (row-group packing).
                q_pair = [per.tile([128 if j < 3 else 64, s], mm_dt,
                                   tag=f"qp{j}", name=f"qp{j}") for j in range(4)]
                qx = [per.tile([128, s], mm_dt, tag=f"qx{j}", name=f"qx{j}")
                      for j in range(4)]
                araw = [per.tile([128 if j < 3 else 64, s], mm_dt,
                                 tag=f"ar{j}", name=f"ar{j}") for j in range(4)]
                # denominator rows live at engine-legal base partitions
                # {0,32,64,96}: head h -> tile h//4, row 32*(h%4)
                den_t = [per.tile([128, s], F32, tag=f"dn{j}", name=f"dn{j}")
                         for j in range(2)]
                rec16 = [per.tile([128, s], rec_dt, tag=f"rc{j}", name=f"rc{j}")
                         for j in range(2)]
                kT2 = per.tile([128, s], mm_dt, tag="kT2", name="kT2")
                k_all = per.tile([128, ST, 64], mm_dt, tag="k_all", name="k_all")
                v_all = per.tile([128, ST, 65], mm_dt, tag="v_all", name="v_all")
                if mm_dt == F16:
                    nc.vector.memset(
                        v_all[:, :, 64:65].bitcast(mybir.dt.uint16), 0x3C00)
                else:
                    nc.vector.memset(v_all[:, :, 64:65].bitcast(F32), 1.0)

                # ============== Phase A: projections + RoPE + transposes =====
                with tc.tile_pool(name="xp", bufs=1) as xp, \
                     tc.tile_pool(name="psA", bufs=1, space="PSUM") as psA:
                    xT_sb = [xp.tile([128, s], mm_dt, tag=f"xT{k}", name=f"xT{k}")
                             for k in range(KCH)]
                    # halves: the st loop's first 8 tiles only need cols
                    # [0, s/2), so projections start ~5us earlier
                    for k in range(KCH):
                        nc.scalar.dma_start(out=xT_sb[k][:, 0:s // 2],
                                            in_=xT[k * 128:(k + 1) * 128,
                                                   0:s // 2])
                    for k in range(KCH):
                        nc.scalar.dma_start(out=xT_sb[k][:, s // 2:s],
                                            in_=xT[k * 128:(k + 1) * 128,
                                                   s // 2:s])
                    _late_loads()
                    _table_casts()
                    def _emit_qtrans(st, q_rot):
                        for cc in range(4):
                            w = 128 if cc < 3 else 64
                            t_ps = psA.tile([128, 128], mm_dt, tag="tr", name="tr",
                                            bufs=4)[0:w, :]
                            nc.tensor.transpose(t_ps, q_rot[:, cc * 128:cc * 128 + w],
                                                idn_mm)
                            nc.scalar.copy(out=q_pair[cc][:, st * 128:(st + 1) * 128],
                                           in_=t_ps)

                    def _emit_a2_chunk(c):
                        # RoPE on k for s-tiles [4c, 4c+4) + PE transpose +
                        # kT2 half-duplication + qx half-swap DMAs for the
                        # matching 512-col q chunk. Emitted inline during the
                        # phase-A loop so the old serial A2 tail disappears.
                        sl = slice(4 * c, 4 * c + 4)
                        kv4 = k_all[:].rearrange("p s (j u) -> p s j u", j=2)
                        ke, ko = kv4[:, sl, 0, :], kv4[:, sl, 1, :]
                        k1 = tmp.tile([128, 4, 32], F32, tag="k1", name="k1", bufs=1)
                        k2 = tmp.tile([128, 4, 32], F32, tag="k2", name="k2", bufs=1)
                        k3 = tmp.tile([128, 4, 32], F32, tag="k3", name="k3", bufs=1)
                        k4 = tmp.tile([128, 4, 32], F32, tag="k4", name="k4", bufs=1)
                        nc.vector.tensor_mul(k1[:], ke, cos_sb[:, sl, :])
                        nc.vector.tensor_mul(k2[:], ko, sin_sb[:, sl, :])
                        nc.vector.tensor_mul(k3[:], ke, sin_sb[:, sl, :])
                        nc.vector.tensor_mul(k4[:], ko, cos_sb[:, sl, :])
                        nc.vector.tensor_sub(ke, k1[:], k2[:])
                        nc.vector.tensor_add(ko, k3[:], k4[:])
                        for st in range(4 * c, 4 * c + 4):
                            t_ps = psA.tile([128, 128], mm_dt, tag="tr",
                                            name="tr", bufs=4)[0:64, :]
                            nc.tensor.transpose(t_ps, k_all[:, st, :], idn_mm)
                            nc.vector.tensor_copy(
                                kT2[0:64, st * 128:(st + 1) * 128], t_ps)
                            nc.vector.tensor_copy(
                                kT2[64:128, st * 128:(st + 1) * 128], t_ps)
                        lo, hi = 512 * c, 512 * (c + 1)
                        for j in range(4):
                            nc.sync.dma_start(out=qx[j][64:128, lo:hi],
                                              in_=q_pair[j][0:64, lo:hi])
                            if j < 3:
                                nc.sync.dma_start(out=qx[j][0:64, lo:hi],
                                                  in_=q_pair[j][64:128, lo:hi])

                    pend_a = []
                    for st in (range(ST) if "A" in phases else []):
                        q_ps = psA.tile([128, GD], F32, tag="q", name="q",
                                        bufs=2)
                        kv_ps = psA.tile([128, 128], F32, tag="kv", name="kv",
                                         bufs=2)
                        for kc in range(KCH):
                            lhsT = xT_sb[kc][:, st * 128:(st + 1) * 128]
                            nc.tensor.matmul(q_ps[:], lhsT,
                                             wq_sb[:, kc, 0:GD],
                                             start=(kc == 0), stop=(kc == KCH - 1))
                            nc.tensor.matmul(kv_ps[:], lhsT,
                                             wq_sb[:, kc, GD:GD + 128],
                                             start=(kc == 0), stop=(kc == KCH - 1))
                        # RoPE on q. Host-side the per-head channels are
                        # permuted to [evens | odds] blocks (scores are
                        # invariant to a shared q/k row permutation), so the
                        # even/odd views here are CONTIGUOUS 32-runs instead
                        # of stride-2 — much friendlier DVE access patterns.
                        # One f32->f16 cast, then all-f16 math at 2x DVE rate.
                        qf = tmp.tile([128, GD], F16, tag="qf", name="qf")
                        nc.scalar.copy(out=qf[:], in_=q_ps[:])
                        qv = qf[:].rearrange("p (h j u) -> p h j u", j=2, u=32)
                        e, o = qv[:, :, 0, :], qv[:, :, 1, :]
                        cb = _bc7(cos16[:, st, :])
                        sb_ = _bc7(sin16[:, st, :])
                        t1 = tmp.tile([128, HL, 32], F16, tag="t1", name="t1", bufs=1)
                        t2 = tmp.tile([128, HL, 32], F16, tag="t2", name="t2", bufs=1)
                        t3 = tmp.tile([128, HL, 32], F16, tag="t3", name="t3", bufs=1)
                        t4 = tmp.tile([128, HL, 32], F16, tag="t4", name="t4", bufs=1)
                        nc.vector.tensor_mul(t1[:], e, cb)
                        nc.vector.tensor_mul(t2[:], o, sb_)
                        nc.vector.tensor_mul(t3[:], e, sb_)
                        nc.vector.tensor_mul(t4[:], o, cb)
                        q_rot = tmp.tile([128, GD], mm_dt, tag="qrot", name="qrot")
                        qrv = q_rot[:].rearrange("p (h j u) -> p h j u", j=2, u=32)
                        nc.vector.tensor_sub(qrv[:, :, 0, :], t1[:], t2[:])
                        nc.vector.tensor_add(qrv[:, :, 1, :], t3[:], t4[:])
                        # k, v staging (k RoPE'd later in one batched pass)
                        nc.vector.tensor_copy(k_all[:, st, :], kv_ps[:, 0:64])
                        nc.vector.tensor_copy(v_all[:, st, 0:64], kv_ps[:, 64:128])
                        # transposes for the PREVIOUS s-tile go after this
                        # tile's projections so PE never waits on the RoPE DVE
                        pend_a.append((st, q_rot))
                        if len(pend_a) > 1:
                            _emit_qtrans(*pend_a.pop(0))
                        if "2" in phases and st in (5, 9, 13):
                            _emit_a2_chunk(st // 4 - 1)

                    for _p in pend_a:
                        _emit_qtrans(*_p)
                    pend_a = []
                    if "2" in phases and "A" in phases:
                        _emit_a2_chunk(3)

                # ================= Phase B: attention =================
                # heads outer (order [4,5,6,0,1,2,3] so den_t[1]'s heads
                # finish early and their normalization overlaps the remaining
                # heads' ACT-bound exp stream), qc inner descending. k-tiles
                # in pairs sharing a [128,1024] psum + one exp; the pair's
                # two scores matmuls run CONCURRENTLY on the PE (row-groups
                # 0-1 / 2-3 via base partitions 0 / 64). Software-pipelined
                # (LOOKP pairs in flight) so PE stays ahead of ACT. Unwritten
                # psum regions of partial (diagonal) tiles hold stale garbage
                # whose exp is never consumed.
                LOOKP = 2

                # The normalize multiplies wait on a DRAM-roundtrip broadcast
                # DMA. Emitting them inline would head-of-line-block the
                # in-order DVE stream (triu masks / psum evacuations) behind
                # that DMA, stalling the exp->AV chain. Instead they are
                # queued as closures and flushed a few (h, qc) iterations
                # later, by which point the DMA has long landed.
                pend_muls = []
                gctr = [0]   # AV-group counter (a mul's "age" clock)

                def flush_muls(n, min_age=0):
                    for _ in range(n):
                        if not pend_muls:
                            return
                        born, fn = pend_muls[0]
                        if gctr[0] - born < min_age:
                            return
                        pend_muls.pop(0)
                        fn()

                def emit_norm(j, heads):
                    # batched softmax normalization for the heads whose
                    # denominator rows live in den_t[j]: fast reciprocal on
                    # DVE (no ACT tables; garbage rows produce garbage,
                    # never read), cast to f16, per-head DRAM-roundtrip
                    # partition-broadcast, then araw *= rec in place.
                    rec32 = tmp.tile([128, s], F32, tag="rec32", name="rec32",
                                     bufs=1)
                    nc.vector.reciprocal_approx_fast(out=rec32[:],
                                                     in_=den_t[j][:])
                    nc.vector.tensor_copy(rec16[j][:], rec32[:])
                    for h in heads:
                        half = (h % 2) * 64
                        row = 32 * (h % 4)
                        nc.sync.dma_start(out=dram_rec[h:h + 1, :],
                                          in_=rec16[j][row:row + 1, :])
                        rbF = tmp.tile([128, s], rec_dt, tag="rbF", name="rbF",
                                       bufs=4)
                        nc.sync.dma_start(out=rbF[:], in_=bass.AP(
                            tensor=dram_rec.tensor, offset=h * s,
                            ap=[[0, 128], [1, s]]))
                        for cq in range(QC):
                            def _mul(h=h, half=half, rbF=rbF, cq=cq):
                                lo, hi = 512 * cq, 512 * (cq + 1)
                                nc.vector.tensor_mul(
                                    araw[h // 2][half:half + 64, lo:hi],
                                    araw[h // 2][half:half + 64, lo:hi],
                                    rbF[half:half + 64, lo:hi])
                            pend_muls.append((gctr[0], _mul))

                HEAD_ORDER = [4, 5, 6, 0, 1, 2, 3]
                with tc.tile_pool(name="psB", bufs=1, space="PSUM") as psB, \
                     tc.tile_pool(name="psO", bufs=2, space="PSUM") as psO, \
                     tc.tile_pool(name="psC", bufs=2, space="PSUM") as psC, \
                     tc.tile_pool(name="expp", bufs=1) as expp:
                    # Flat global pipeline over all (head, qc, k-tile-group)
                    # units: a sliding window keeps LOOK groups of scores+exp
                    # in flight ACROSS unit boundaries, so the ACT exp stream
                    # never drains at head/qc transitions. Deferred work
                    # (normalize multiplies, output-projection passes) is
                    # age-gated and drip-fed between AV groups: it only
                    # enters an engine's in-order stream once its upstream
                    # DMA roundtrip has surely landed.
                    sched = []
                    if "B" in phases:
                        # qc outer (ascending): each qc's 7 heads complete
                        # mid-stream, so normalization + output projection
                        # for finished qcs drip into the pipeline while
                        # later (bigger) qcs keep ACT saturated
                        for qc in range(QC):
                            for h in range(HL):
                                sched.append((h, qc))
                    gseq = []
                    for ui, (h, qc) in enumerate(sched):
                        nkt = 4 * (qc + 1)
                        for k0 in range(0, nkt, 2):
                            gseq.append((ui, list(range(k0, min(k0 + 2, nkt)))))
                    unit_ops = {}
                    pend = {}

                    def emit_scores(n):
                        ui, kts = gseq[n]
                        h, qc = sched[ui]
                        q_lo = q_pair[h // 2] if h % 2 == 0 else qx[h // 2]
                        q_hi = qx[h // 2] if h % 2 == 0 else q_pair[h // 2]
                        s_ps = psB.tile([128, 1024], F32, tag="sp",
                                        name="sp", bufs=2)
                        ex = expp.tile([128, 1024], mm_dt, tag="ex",
                                       name="ex", bufs=6)
                        info = []
                        for idx, kt in enumerate(kts):
                            rrel = kt - 4 * qc
                            off = 128 * rrel if rrel >= 0 else 0
                            N = 512 - off
                            hf = (kt % 2) * 64
                            qsrc = q_lo if kt % 2 == 0 else q_hi
                            nc.tensor.matmul(
                                s_ps[:, 512 * idx + off:512 * (idx + 1)],
                                kT2[hf:hf + 64, kt * 128:(kt + 1) * 128],
                                qsrc[hf:hf + 64,
                                     qc * 512 + off:(qc + 1) * 512],
                                start=True, stop=True)
                            info.append((kt, 512 * idx + off, off, N, rrel))
                        # exp over each contiguous written run (a diagonal
                        # non-first tile leaves an unwritten gap)
                        runs = []
                        for kt, base, off, N, rrel in info:
                            if runs and runs[-1][1] == base:
                                runs[-1][1] = base + N
                            else:
                                runs.append([base, base + N])
                        for lo, hi in runs:
                            nc.scalar.activation(out=ex[:, lo:hi],
                                                 in_=s_ps[:, lo:hi],
                                                 func=AF.Exp)
                        for kt, base, off, N, rrel in info:
                            if rrel >= 0:
                                nc.vector.tensor_mul(
                                    ex[:, base:base + 128],
                                    ex[:, base:base + 128], triu_sb[:])
                        pend[n] = (ex, info)

                    def emit_ypass(st, c0, cn):
                        y_ps = psC.tile([128, 512], F32, tag="y", name="y")
                        for cc in range(4):
                            w = 128 if cc < 3 else 64
                            lhsT = araw[cc][0:w, st * 128:(st + 1) * 128]
                            nc.tensor.matmul(y_ps[:, 0:cn], lhsT,
                                             wo_sb[0:w, cc, c0:c0 + cn],
                                             start=(cc == 0), stop=(cc == 3))
                        y_sb = tmp.tile([128, 512], F16, tag="ysb",
                                        name="ysb", bufs=4)
                        nc.vector.tensor_copy(y_sb[:, 0:cn], y_ps[:, 0:cn])
                        nc.sync.dma_start(
                            out=y[st * 128:(st + 1) * 128, c0:c0 + cn],
                            in_=y_sb[:, 0:cn])

                    pend_ypass = []

                    def flush_ypass(n, min_age=0):
                        for _ in range(n):
                            if not pend_ypass:
                                return
                            born, fn = pend_ypass[0]
                            if gctr[0] - born < min_age:
                                return
                            # CORRECTNESS: the qc's normalize multiplies must
                            # be emitted before its projection reads araw
                            if pend_muls and pend_muls[0][0] <= born:
                                return
                            pend_ypass.pop(0)
                            fn()

                    def unit_done(ui):
                        # evacuate o_ps + deferred-normalization hooks
                        h, qc = sched[ui]
                        half = (h % 2) * 64
                        o_ps = unit_ops.pop(ui)
                        nc.vector.tensor_copy(
                            araw[h // 2][half:half + 64,
                                         qc * 512:(qc + 1) * 512],
                            o_ps[0:64, :])
                        row = 32 * (h % 4)
                        nc.vector.tensor_copy(
                            den_t[h // 4][row:row + 1,
                                          qc * 512:(qc + 1) * 512],
                            o_ps[64:65, :])
                        if h == HL - 1:
                            # all 7 heads done with this qc: normalize both
                            # den tiles' chunks, then queue the qc's output
                            # projection (age-gated deeper than the muls)
                            qlo, qhi = qc * 512, (qc + 1) * 512
                            for j in (0, 1):
                                rec32 = tmp.tile([128, 512], F32,
                                                 tag="rec32c", name="rec32c",
                                                 bufs=2)
                                nc.vector.reciprocal_approx_fast(
                                    out=rec32[:], in_=den_t[j][:, qlo:qhi])
                                nc.vector.tensor_copy(rec16[j][:, qlo:qhi],
                                                      rec32[:])
                            for hh in range(HL):
                                hhalf = (hh % 2) * 64
                                jj, rw = hh // 4, 32 * (hh % 4)
                                nc.sync.dma_start(
                                    out=dram_rec[hh:hh + 1, qlo:qhi],
                                    in_=rec16[jj][rw:rw + 1, qlo:qhi])
                                rbF = tmp.tile([128, 512], rec_dt,
                                               tag="rbFc", name="rbFc",
                                               bufs=8)
                                nc.sync.dma_start(out=rbF[:], in_=bass.AP(
                                    tensor=dram_rec.tensor,
                                    offset=hh * s + qlo,
                                    ap=[[0, 128], [1, 512]]))

                                def _mul(hh=hh, hhalf=hhalf, qlo=qlo,
                                         qhi=qhi, rbF=rbF):
                                    nc.vector.tensor_mul(
                                        araw[hh // 2][hhalf:hhalf + 64,
                                                      qlo:qhi],
                                        araw[hh // 2][hhalf:hhalf + 64,
                                                      qlo:qhi],
                                        rbF[hhalf:hhalf + 64, :])
                                pend_muls.append((gctr[0], _mul))
                            if "C" in phases:
                                for st in range(4 * qc, 4 * qc + 4):
                                    for c0, cn in ((0, 512), (512, 384)):
                                        def _yp(st=st, c0=c0, cn=cn):
                                            emit_ypass(st, c0, cn)
                                        pend_ypass.append((gctr[0], _yp))

                    LOOK = 3
                    NG = len(gseq)
                    for n in range(min(LOOK, NG)):
                        emit_scores(n)
                    for n in range(NG):
                        # AV first, THEN the lookahead scores: the deeper
                        # lookahead's psum wait can't head-of-line block the
                        # ready AV group in the in-order PE stream
                        ui, kts = gseq[n]
                        h, qc = sched[ui]
                        nkt = 4 * (qc + 1)
                        if ui not in unit_ops:
                            unit_ops[ui] = psO.tile([65, 512], F32, tag="o",
                                                    name="o")
                        o_ps = unit_ops[ui]
                        ex, info = pend.pop(n)
                        for kt, base, off, N, rrel in info:
                            nc.tensor.matmul(
                                o_ps[:, off:512], v_all[:, kt, :],
                                ex[:, base:base + N],
                                start=(kt == 0), stop=(kt == nkt - 1))
                        if n + LOOK < NG:
                            emit_scores(n + LOOK)
                        gctr[0] += 1
                        flush_muls(3, min_age=6)
                        flush_ypass(1, min_age=10)
                        if kts[-1] == nkt - 1:
                            unit_done(ui)
                    flush_muls(len(pend_muls))
                    flush_ypass(len(pend_ypass))
                    if "C" not in phases or "B" not in phases:
                        y_sb = tmp.tile([128, 512], F16, tag="ysb", name="ysb",
                                        bufs=4)
                        nc.vector.memset(y_sb[:], 0.0)
                        nc.sync.dma_start(out=y[0:128, 0:512], in_=y_sb[:])

            if reps > 1:
                with tc.For_i(0, reps, 1):
                    _body()
            else:
                _body()

    nc.compile()
    return nc


# ---------------------------------------------------------------------------
# host-side sharding + execution
# ---------------------------------------------------------------------------

def round_f32r(a):
    """Round fp32 array to fp32r (RNE to 11 mantissa bits) -- bit-exact match
    of the hardware's casting DMA, verified by SBUF readback."""
    b = np.ascontiguousarray(a, dtype=np.float32).view(np.uint32)
    lsb = (b >> np.uint32(12)) & np.uint32(1)
    r = ((b + np.uint32(0x7FF) + lsb) & np.uint32(0xFFFFF000))
    return r.view(np.float32)


MM_DT = {"f32r": F32R, "f16": F16, "f32": F32}[os.environ.get("MM_DT", "f16")]


def _cvt(a, mm_dt):
    if mm_dt == F16:
        return np.ascontiguousarray(np.asarray(a, dtype=np.float32)).astype(np.float16)
    if mm_dt == F32R:
        return round_f32r(a)
    return np.ascontiguousarray(a, dtype=np.float32)


def make_in_maps(x, freqs_cos, freqs_sin, Wq, Wk, Wv, Wo, s=S, mm_dt=None):
    if mm_dt is None:
        mm_dt = MM_DT
    ST = s // 128
    scale = 1.0 / math.sqrt(D)
    cosr = np.ascontiguousarray(
        np.asarray(freqs_cos).reshape(ST, 128, 32).transpose(1, 0, 2)).astype(np.float32)
    sinr = np.ascontiguousarray(
        np.asarray(freqs_sin).reshape(ST, 128, 32).transpose(1, 0, 2)).astype(np.float32)
    triu = _cvt(np.triu(np.ones((128, 128), dtype=np.float32)), mm_dt)
    # per-head channel permutation [evens | odds] applied to q and k
    # (RoPE pairs become contiguous blocks on-device; scores are invariant)
    perm = np.concatenate([np.arange(0, D, 2), np.arange(1, D, 2)])
    qperm = np.concatenate([h * D + perm for h in range(HL)])
    in_maps = []
    for c in range(N_CORES):
        b, g = c // 2, c % 2
        in_maps.append({
            "xT": _cvt(np.asarray(x)[b].T, mm_dt),
            "wq": _cvt(np.concatenate(
                [np.asarray(Wq)[:, g * GD:(g + 1) * GD][:, qperm] * scale,
                 np.asarray(Wk)[:, g * D:(g + 1) * D][:, perm],
                 np.asarray(Wv)[:, g * D:(g + 1) * D]], axis=1), mm_dt),
            "wo": _cvt(np.asarray(Wo)[g * GD:(g + 1) * GD, :], mm_dt),
            "cosr": cosr, "sinr": sinr, "triu": triu,
        })
    return in_maps


_RUNNER = None


class _Runner:
    """Minimal SPMD executor over axon PJRT (self-contained copy)."""

    def __init__(self, nc, n_cores):
        import jax
        from jax.sharding import Mesh, PartitionSpec, NamedSharding
        from jax.experimental.shard_map import shard_map
        from concourse.bass2jax import (_bass_exec_p, install_neuronx_cc_hook,
                                        partition_id_tensor)
        install_neuronx_cc_hook()
        self.jax = jax
        self.n_cores = n_cores
        partition_name = (nc.partition_id_tensor.name
                          if nc.partition_id_tensor else None)
        in_names, out_names, out_avals = [], [], []
        for alloc in nc.m.functions[0].allocations:
            if not isinstance(alloc, mybir.MemoryLocationSet):
                continue
            name = alloc.memorylocations[0].name
            if alloc.kind == "ExternalInput":
                if name != partition_name:
                    in_names.append(name)
            elif alloc.kind == "ExternalOutput":
                out_names.append(name)
                out_avals.append(jax.core.ShapedArray(
                    tuple(alloc.tensor_shape), mybir.dt.np(alloc.dtype)))
        self.in_names, self.out_names, self.out_avals = in_names, out_names, out_avals
        n_params, n_outs = len(in_names), len(out_avals)
        all_names = in_names + out_names
        if partition_name is not None:
            all_names.append(partition_name)

        def _body(*args):
            operands = list(args)
            if partition_name is not None:
                operands.append(partition_id_tensor())
            return tuple(_bass_exec_p.bind(
                *operands, out_avals=tuple(out_avals), in_names=tuple(all_names),
                out_names=tuple(out_names), lowering_input_output_aliases=(),
                sim_require_finite=False, sim_require_nnan=False, nc=nc))

        devices = jax.devices()[:n_cores]
        self.mesh = Mesh(np.asarray(devices), ("core",))
        self.sharding = NamedSharding(self.mesh, PartitionSpec("core"))
        in_specs = (PartitionSpec("core"),) * (n_params + n_outs)
        out_specs = (PartitionSpec("core"),) * n_outs
        self.fn = jax.jit(
            shard_map(_body, mesh=self.mesh, in_specs=in_specs,
                      out_specs=out_specs, check_rep=False),
            donate_argnums=tuple(range(n_params, n_params + n_outs)),
            keep_unused=True)
        zshapes = [(n_cores * a.shape[0], *a.shape[1:]) for a in out_avals]
        zdtypes = [a.dtype for a in out_avals]
        self.make_zeros = jax.jit(
            lambda: tuple(jax.numpy.zeros(sh, dt)
                          for sh, dt in zip(zshapes, zdtypes)),
            out_shardings=tuple(self.sharding for _ in zshapes))

    def prep(self, in_maps):
        return [self.jax.device_put(
            np.concatenate([np.asarray(in_maps[c][n]) for c in range(self.n_cores)],
                           axis=0), self.sharding)
            for n in self.in_names]

    def run(self, dev_in):
        return self.fn(*dev_in, *self.make_zeros())

    def split(self, outs):
        res = []
        for c in range(self.n_cores):
            res.append({n: np.asarray(outs[i]).reshape(
                self.n_cores, *self.out_avals[i].shape)[c]
                for i, n in enumerate(self.out_names)})
        return res


def get_runner():
    global _RUNNER
    if _RUNNER is None:
        _RUNNER = _Runner(build(), N_CORES)
    return _RUNNER


def kernel(x, freqs_cos, freqs_sin, mask, Wq, Wk, Wv, Wo):
    x = np.asarray(x, dtype=np.float32)
    in_maps = make_in_maps(np.asarray(x), np.asarray(freqs_cos),
                           np.asarray(freqs_sin), np.asarray(Wq),
                           np.asarray(Wk), np.asarray(Wv), np.asarray(Wo))
    r = get_runner()
    outs = r.run(r.prep(in_maps))
    res = r.split(outs)
    out = np.empty((B, S, HID), dtype=np.float32)
    for b in range(B):
        out[b] = (res[2 * b]["y"].astype(np.float32)
                  + res[2 * b + 1]["y"].astype(np.float32))
    return out


# revision 66
# speedup vs baseline: 1.1039x; 1.1039x over previous
"""Trainium2 Bass kernel for GQA attention (B=4, S=2048, HID=896, H=14, KV=2, D=64).

Sharding: 8 cores = 4 batches x 2 KV-head groups. Core c handles batch c//2,
query heads [g*7, (g+1)*7) with g = c%2 (exactly one KV head per core thanks to
GQA group structure). Each core computes its 448-channel slice of attn output
and the partial output projection y_g = ao_g @ Wo[g*448:(g+1)*448, :]; the host
sums the two partials per batch (f16 partials, f32 host sum).

Per-core pipeline (S=2048):
  A)  q = x@Wq_g (448 ch), kv = x@[Wk|Wv]_g (128 ch) from one fused weight
      tile, lhsT = x^T tiles (loaded in s-halves so compute starts early).
      RoPE on DVE in f16 at 2x rate: the per-head channels are host-permuted
      to [evens | odds] blocks so the even/odd views are contiguous (scores
      are invariant to a shared q/k row permutation). PE-transpose q and k to
      channel-major; k RoPE + transposes + kT/q half-duplication emitted in
      4-s-tile chunks inline with the loop (no serial tail). kT2 holds k in
      both partition halves; q_pair holds heads (2j, 2j+1) at halves (0, 64)
      and qx the half-swap, so every head's scores can issue from EITHER PE
      row-group half.
  B+C) One flat software-pipelined stream over all (qc, head, k-tile-pair)
      units, qc outer ascending. scoresT[k,q] = kT.T @ qT (K=64): the pair's
      two matmuls run CONCURRENTLY on the PE (row-groups 0-1 / 2-3 via base
      partitions 0 / 64, ~512 cycles per packed pair); one [128,1024] PSUM
      tile + one exp per pair keeps ACT ~95% saturated -- the exp stream is
      the kernel's critical path. Structural causality (only k-tiles <=
      diagonal; diagonal tiles masked with a triangular 0/1 multiply after
      exp). attn@[v|1] accumulates into PSUM [65, 512]; row 64 = softmax
      denominators. After a qc's 7 heads finish: 1/den via the custom-DVE
      fast reciprocal (no ACT tables, no Ln/Exp), DRAM-roundtrip partition
      broadcast, in-place normalize of the raw attn rows, then the qc's
      output projection y = ao @ Wo_g (split 512/384 columns, [128,512] PSUM)
      -> f16 y. The normalize multiplies and projection passes are AGE-GATED
      deferred work, drip-fed into the engine streams a few pipeline steps
      after their DMA roundtrips were issued, so the in-order DVE/PE streams
      never head-of-line block on a DMA (projections additionally wait until
      their qc's normalize multiplies are emitted -- correctness).

The causal mask input is never loaded: exp(-1e9 + s) == 0.0 exactly in fp32,
so structural masking matches the reference's additive mask bit-for-bit.
PSUM budget: scores 2x[128,1024] (4 banks) + attn@v 2x[65,512] (2) + output
projection 2x[128,512] (2) = 8 banks.

mm_dt selects the matmul dtype: float32r (2 PE cycles/row, ~1.4e-4 rounding =
RNE to 11 mantissa bits, replicated host-side) or float16 (1 cycle/row, ~2x
the rounding error) or float32 (exact, 4 cycles/row).
"""
import math
import os
import numpy as np

import concourse.bass as bass
import concourse.mybir as mybir
import concourse.tile as tile
from concourse import bacc
from concourse.masks import make_identity

F32 = mybir.dt.float32
F32R = mybir.dt.float32r
F16 = mybir.dt.float16
AF = mybir.ActivationFunctionType

B, S, HID = 4, 2048, 896
H, KV, D = 14, 2, 64
HL = H // KV          # 7 local query heads per core
GD = HL * D           # 448 local channels
KCH = HID // 128      # 7 contraction chunks
N_CORES = 8


def _bc7(ap_small):
    """[128, 32] cos/sin slice -> broadcast over the 7 heads: [128, 7, 32]."""
    return bass.AP(
        tensor=ap_small.tensor,
        offset=ap_small.offset,
        ap=[list(ap_small.ap[0]), [0, HL], list(ap_small.ap[1])],
    )


def build(s=S, mm_dt=None, reps=1, phases="A2BC"):
    if mm_dt is None:
        mm_dt = MM_DT
    ST = s // 128           # s-tiles
    QC = s // 512           # q chunks
    nc = bacc.Bacc("TRN2", target_bir_lowering=False, debug=False,
                   num_devices=N_CORES)

    xT = nc.dram_tensor("xT", [HID, s], mm_dt, kind="ExternalInput").ap()
    wq = nc.dram_tensor("wq", [HID, GD + 128], mm_dt, kind="ExternalInput").ap()
    wo = nc.dram_tensor("wo", [GD, HID], mm_dt, kind="ExternalInput").ap()
    cosr = nc.dram_tensor("cosr", [128, ST, 32], F32, kind="ExternalInput").ap()
    sinr = nc.dram_tensor("sinr", [128, ST, 32], F32, kind="ExternalInput").ap()
    triu = nc.dram_tensor("triu", [128, 128], mm_dt, kind="ExternalInput").ap()
    y = nc.dram_tensor("y", [s, HID], F16, kind="ExternalOutput").ap()
    rec_dt = F16
    dram_rec = nc.dram_tensor("dram_rec", [HL, s], rec_dt).ap()

    with tile.TileContext(nc) as tc:
        with (
            tc.tile_pool(name="wp", bufs=1) as wp,
            tc.tile_pool(name="per", bufs=1) as per,
            tc.tile_pool(name="tmp", bufs=2) as tmp,
        ):
            def _body():
                # ---- small loads (weights/tables stay resident) ----
                # DMA emission order = queue order: wq/wkv first (phase A
                # needs them immediately; xT follows in phase A's pool), then
                # tables, then wo LAST (only phase C reads it) so the
                # DMA-bound lead-in gates on ~4.4MB instead of ~5.7MB.
                # input loads issue from the ACT engine's HW-DGE queue: a
                # separate DMA FIFO from the sync queue that carries the
                # previous rep's tail stores, and ACT's instruction stream
                # frees at the previous rep's last exp -- so the next rep's
                # transfers start ~25us earlier in the reps loop
                wq_sb = wp.tile([128, KCH, GD + 128], mm_dt, tag="wq", name="wq")
                nc.scalar.dma_start(out=wq_sb[:], in_=wq.rearrange("(k p) m -> p k m", p=128))
                cos_sb = wp.tile([128, ST, 32], F32, tag="cos", name="cos")
                sin_sb = wp.tile([128, ST, 32], F32, tag="sin", name="sin")
                triu_sb = wp.tile([128, 128], mm_dt, tag="triu", name="triu")
                wo_sb = wp.tile([128, 4, HID], mm_dt, tag="wo", name="wo")

                def _late_loads():
                    nc.scalar.dma_start(out=cos_sb[:], in_=cosr)
                    nc.scalar.dma_start(out=sin_sb[:], in_=sinr)
                    nc.sync.dma_start(out=triu_sb[:], in_=triu)
                    for cc in range(4):
                        w = 128 if cc < 3 else 64
                        nc.sync.dma_start(out=wo_sb[0:w, cc, :],
                                          in_=wo[cc * 128:cc * 128 + w, :])
                # f16 copies of the RoPE tables for the 2x-rate q RoPE
                cos16 = wp.tile([128, ST, 32], F16, tag="cos16", name="cos16")
                sin16 = wp.tile([128, ST, 32], F16, tag="sin16", name="sin16")

                def _table_casts():
                    nc.vector.tensor_copy(cos16[:], cos_sb[:])
                    nc.vector.tensor_copy(sin16[:], sin_sb[:])

                # PE warmup (single-shot only: in the reps loop the HAM
                # clock-gate is already warm): dummy matmuls on a memset tile
                # start immediately and overlap the input-DMA lead-in
                wdum = wp.tile([128, 128], mm_dt, tag="wdum", name="wdum")
                nc.vector.memset(wdum[:], 0.0)
                with tc.tile_pool(name="psW", bufs=1, space="PSUM") as psW:
                    wps = psW.tile([128, 128], F32, tag="w", name="w")
                    for _ in range(72 if reps == 1 else 0):
                        nc.tensor.matmul(wps[:], wdum[:], wdum[:],
                                         start=True, stop=True)
                idn = wp.tile([128, 128], F32, tag="idn", name="idn")
                make_identity(nc, idn[:])
                idn_r = wp.tile([128, 128], mm_dt, tag="idnr", name="idnr")
                nc.vector.tensor_copy(idn_r[:], idn[:])
                idn_mm = idn_r[:]

                # ---- persistent intermediates ----
                # qT in head-pair chunks: q_pair[j] holds heads (2j, 2j+1) at
                # halves (0, 64); qx[j] is the half-swap of q_pair[j] so
                # every head has its q at BOTH halves (row-group packing).
                q_pair = [per.tile([128 if j < 3 else 64, s], mm_dt,
                                   tag=f"qp{j}", name=f"qp{j}") for j in range(4)]
                qx = [per.tile([128, s], mm_dt, tag=f"qx{j}", name=f"qx{j}")
                      for j in range(4)]
                araw = [per.tile([128 if j < 3 else 64, s], mm_dt,
                                 tag=f"ar{j}", name=f"ar{j}") for j in range(4)]
                # denominator rows live at engine-legal base partitions
                # {0,32,64,96}: head h -> tile h//4, row 32*(h%4)
                den_t = [per.tile([128, s], F32, tag=f"dn{j}", name=f"dn{j}")
                         for j in range(2)]
                rec16 = [per.tile([128, s], rec_dt, tag=f"rc{j}", name=f"rc{j}")
                         for j in range(2)]
                kT2 = per.tile([128, s], mm_dt, tag="kT2", name="kT2")
                k_all = per.tile([128, ST, 64], mm_dt, tag="k_all", name="k_all")
                v_all = per.tile([128, ST, 65], mm_dt, tag="v_all", name="v_all")
                if mm_dt == F16:
                    nc.vector.memset(
                        v_all[:, :, 64:65].bitcast(mybir.dt.uint16), 0x3C00)
                else:
                    nc.vector.memset(v_all[:, :, 64:65].bitcast(F32), 1.0)

                # ============== Phase A: projections + RoPE + transposes =====
                with tc.tile_pool(name="xp", bufs=1) as xp, \
                     tc.tile_pool(name="psA", bufs=1, space="PSUM") as psA:
                    xT_sb = [xp.tile([128, s], mm_dt, tag=f"xT{k}", name=f"xT{k}")
                             for k in range(KCH)]
                    # halves: the st loop's first 8 tiles only need cols
                    # [0, s/2), so projections start ~5us earlier
                    for k in range(KCH):
                        nc.scalar.dma_start(out=xT_sb[k][:, 0:s // 2],
                                            in_=xT[k * 128:(k + 1) * 128,
                                                   0:s // 2])
                    for k in range(KCH):
                        nc.scalar.dma_start(out=xT_sb[k][:, s // 2:s],
                                            in_=xT[k * 128:(k + 1) * 128,
                                                   s // 2:s])
                    _late_loads()
                    _table_casts()
                    def _emit_qtrans(st, q_rot):
                        for cc in range(4):
                            w = 128 if cc < 3 else 64
                            t_ps = psA.tile([128, 128], mm_dt, tag="tr", name="tr",
                                            bufs=4)[0:w, :]
                            nc.tensor.transpose(t_ps, q_rot[:, cc * 128:cc * 128 + w],
                                                idn_mm)
                            nc.scalar.copy(out=q_pair[cc][:, st * 128:(st + 1) * 128],
                                           in_=t_ps)

                    def _emit_a2_chunk(c):
                        # RoPE on k for s-tiles [4c, 4c+4) + PE transpose +
                        # kT2 half-duplication + qx half-swap DMAs for the
                        # matching 512-col q chunk. Emitted inline during the
                        # phase-A loop so the old serial A2 tail disappears.
                        sl = slice(4 * c, 4 * c + 4)
                        kv4 = k_all[:].rearrange("p s (j u) -> p s j u", j=2)
                        ke, ko = kv4[:, sl, 0, :], kv4[:, sl, 1, :]
                        k1 = tmp.tile([128, 4, 32], F32, tag="k1", name="k1", bufs=1)
                        k2 = tmp.tile([128, 4, 32], F32, tag="k2", name="k2", bufs=1)
                        k3 = tmp.tile([128, 4, 32], F32, tag="k3", name="k3", bufs=1)
                        k4 = tmp.tile([128, 4, 32], F32, tag="k4", name="k4", bufs=1)
                        nc.vector.tensor_mul(k1[:], ke, cos_sb[:, sl, :])
                        nc.vector.tensor_mul(k2[:], ko, sin_sb[:, sl, :])
                        nc.vector.tensor_mul(k3[:], ke, sin_sb[:, sl, :])
                        nc.vector.tensor_mul(k4[:], ko, cos_sb[:, sl, :])
                        nc.vector.tensor_sub(ke, k1[:], k2[:])
                        nc.vector.tensor_add(ko, k3[:], k4[:])
                        for st in range(4 * c, 4 * c + 4):
                            t_ps = psA.tile([128, 128], mm_dt, tag="tr",
                                            name="tr", bufs=4)[0:64, :]
                            nc.tensor.transpose(t_ps, k_all[:, st, :], idn_mm)
                            nc.vector.tensor_copy(
                                kT2[0:64, st * 128:(st + 1) * 128], t_ps)
                            nc.vector.tensor_copy(
                                kT2[64:128, st * 128:(st + 1) * 128], t_ps)
                        lo, hi = 512 * c, 512 * (c + 1)
                        for j in range(4):
                            nc.sync.dma_start(out=qx[j][64:128, lo:hi],
                                              in_=q_pair[j][0:64, lo:hi])
                            if j < 3:
                                nc.sync.dma_start(out=qx[j][0:64, lo:hi],
                                                  in_=q_pair[j][64:128, lo:hi])

                    pend_a = []
                    for st in (range(ST) if "A" in phases else []):
                        q_ps = psA.tile([128, GD], F32, tag="q", name="q",
                                        bufs=2)
                        kv_ps = psA.tile([128, 128], F32, tag="kv", name="kv",
                                         bufs=2)
                        for kc in range(KCH):
                            lhsT = xT_sb[kc][:, st * 128:(st + 1) * 128]
                            nc.tensor.matmul(q_ps[:], lhsT,
                                             wq_sb[:, kc, 0:GD],
                                             start=(kc == 0), stop=(kc == KCH - 1))
                            nc.tensor.matmul(kv_ps[:], lhsT,
                                             wq_sb[:, kc, GD:GD + 128],
                                             start=(kc == 0), stop=(kc == KCH - 1))
                        # RoPE on q. Host-side the per-head channels are
                        # permuted to [evens | odds] blocks (scores are
                        # invariant to a shared q/k row permutation), so the
                        # even/odd views here are CONTIGUOUS 32-runs instead
                        # of stride-2 — much friendlier DVE access patterns.
                        # One f32->f16 cast, then all-f16 math at 2x DVE rate.
                        qf = tmp.tile([128, GD], F16, tag="qf", name="qf")
                        nc.scalar.copy(out=qf[:], in_=q_ps[:])
                        qv = qf[:].rearrange("p (h j u) -> p h j u", j=2, u=32)
                        e, o = qv[:, :, 0, :], qv[:, :, 1, :]
                        cb = _bc7(cos16[:, st, :])
                        sb_ = _bc7(sin16[:, st, :])
                        t1 = tmp.tile([128, HL, 32], F16, tag="t1", name="t1", bufs=1)
                        t2 = tmp.tile([128, HL, 32], F16, tag="t2", name="t2", bufs=1)
                        t3 = tmp.tile([128, HL, 32], F16, tag="t3", name="t3", bufs=1)
                        t4 = tmp.tile([128, HL, 32], F16, tag="t4", name="t4", bufs=1)
                        nc.vector.tensor_mul(t1[:], e, cb)
                        nc.vector.tensor_mul(t2[:], o, sb_)
                        nc.vector.tensor_mul(t3[:], e, sb_)
                        nc.vector.tensor_mul(t4[:], o, cb)
                        q_rot = tmp.tile([128, GD], mm_dt, tag="qrot", name="qrot")
                        qrv = q_rot[:].rearrange("p (h j u) -> p h j u", j=2, u=32)
                        nc.vector.tensor_sub(qrv[:, :, 0, :], t1[:], t2[:])
                        nc.vector.tensor_add(qrv[:, :, 1, :], t3[:], t4[:])
                        # k, v staging (k RoPE'd later in one batched pass)
                        nc.vector.tensor_copy(k_all[:, st, :], kv_ps[:, 0:64])
                        nc.vector.tensor_copy(v_all[:, st, 0:64], kv_ps[:, 64:128])
                        # transposes for the PREVIOUS s-tile go after this
                        # tile's projections so PE never waits on the RoPE DVE
                        pend_a.append((st, q_rot))
                        if len(pend_a) > 1:
                            _emit_qtrans(*pend_a.pop(0))
                        if "2" in phases and st in (5, 9, 13):
                            _emit_a2_chunk(st // 4 - 1)

                    for _p in pend_a:
                        _emit_qtrans(*_p)
                    pend_a = []
                    if "2" in phases and "A" in phases:
                        _emit_a2_chunk(3)

                # ================= Phase B: attention =================
                # heads outer (order [4,5,6,0,1,2,3] so den_t[1]'s heads
                # finish early and their normalization overlaps the remaining
                # heads' ACT-bound exp stream), qc inner descending. k-tiles
                # in pairs sharing a [128,1024] psum + one exp; the pair's
                # two scores matmuls run CONCURRENTLY on the PE (row-groups
                # 0-1 / 2-3 via base partitions 0 / 64). Software-pipelined
                # (LOOKP pairs in flight) so PE stays ahead of ACT. Unwritten
                # psum regions of partial (diagonal) tiles hold stale garbage
                # whose exp is never consumed.
                LOOKP = 2

                # The normalize multiplies wait on a DRAM-roundtrip broadcast
                # DMA. Emitting them inline would head-of-line-block the
                # in-order DVE stream (triu masks / psum evacuations) behind
                # that DMA, stalling the exp->AV chain. Instead they are
                # queued as closures and flushed a few (h, qc) iterations
                # later, by which point the DMA has long landed.
                pend_muls = []
                gctr = [0]   # AV-group counter (a mul's "age" clock)

                def flush_muls(n, min_age=0):
                    for _ in range(n):
                        if not pend_muls:
                            return
                        born, fn = pend_muls[0]
                        if gctr[0] - born < min_age:
                            return
                        pend_muls.pop(0)
                        fn()

                def emit_norm(j, heads):
                    # batched softmax normalization for the heads whose
                    # denominator rows live in den_t[j]: fast reciprocal on
                    # DVE (no ACT tables; garbage rows produce garbage,
                    # never read), cast to f16, per-head DRAM-roundtrip
                    # partition-broadcast, then araw *= rec in place.
                    rec32 = tmp.tile([128, s], F32, tag="rec32", name="rec32",
                                     bufs=1)
                    nc.vector.reciprocal_approx_fast(out=rec32[:],
                                                     in_=den_t[j][:])
                    nc.vector.tensor_copy(rec16[j][:], rec32[:])
                    for h in heads:
                        half = (h % 2) * 64
                        row = 32 * (h % 4)
                        nc.sync.dma_start(out=dram_rec[h:h + 1, :],
                                          in_=rec16[j][row:row + 1, :])
                        rbF = tmp.tile([128, s], rec_dt, tag="rbF", name="rbF",
                                       bufs=4)
                        nc.sync.dma_start(out=rbF[:], in_=bass.AP(
                            tensor=dram_rec.tensor, offset=h * s,
                            ap=[[0, 128], [1, s]]))
                        for cq in range(QC):
                            def _mul(h=h, half=half, rbF=rbF, cq=cq):
                                lo, hi = 512 * cq, 512 * (cq + 1)
                                nc.vector.tensor_mul(
                                    araw[h // 2][half:half + 64, lo:hi],
                                    araw[h // 2][half:half + 64, lo:hi],
                                    rbF[half:half + 64, lo:hi])
                            pend_muls.append((gctr[0], _mul))

                HEAD_ORDER = [4, 5, 6, 0, 1, 2, 3]
                with tc.tile_pool(name="psB", bufs=1, space="PSUM") as psB, \
                     tc.tile_pool(name="psO", bufs=2, space="PSUM") as psO, \
                     tc.tile_pool(name="psC", bufs=2, space="PSUM") as psC, \
                     tc.tile_pool(name="expp", bufs=1) as expp:
                    # Flat global pipeline over all (head, qc, k-tile-group)
                    # units: a sliding window keeps LOOK groups of scores+exp
                    # in flight ACROSS unit boundaries, so the ACT exp stream
                    # never drains at head/qc transitions. Deferred work
                    # (normalize multiplies, output-projection passes) is
                    # age-gated and drip-fed between AV groups: it only
                    # enters an engine's in-order stream once its upstream
                    # DMA roundtrip has surely landed.
                    sched = []
                    if "B" in phases:
                        # qc outer (ascending): each qc's 7 heads complete
                        # mid-stream, so normalization + output projection
                        # for finished qcs drip into the pipeline while
                        # later (bigger) qcs keep ACT saturated
                        for qc in range(QC):
                            for h in range(HL):
                                sched.append((h, qc))
                    gseq = []
                    for ui, (h, qc) in enumerate(sched):
                        nkt = 4 * (qc + 1)
                        for k0 in range(0, nkt, 2):
                            gseq.append((ui, list(range(k0, min(k0 + 2, nkt)))))
                    unit_ops = {}
                    pend = {}

                    def emit_scores(n):
                        ui, kts = gseq[n]
                        h, qc = sched[ui]
                        q_lo = q_pair[h // 2] if h % 2 == 0 else qx[h // 2]
                        q_hi = qx[h // 2] if h % 2 == 0 else q_pair[h // 2]
                        s_ps = psB.tile([128, 1024], F32, tag="sp",
                                        name="sp", bufs=2)
                        ex = expp.tile([128, 1024], mm_dt, tag="ex",
                                       name="ex", bufs=4)
                        info = []
                        for idx, kt in enumerate(kts):
                            rrel = kt - 4 * qc
                            off = 128 * rrel if rrel >= 0 else 0
                            N = 512 - off
                            hf = (kt % 2) * 64
                            qsrc = q_lo if kt % 2 == 0 else q_hi
                            nc.tensor.matmul(
                                s_ps[:, 512 * idx + off:512 * (idx + 1)],
                                kT2[hf:hf + 64, kt * 128:(kt + 1) * 128],
                                qsrc[hf:hf + 64,
                                     qc * 512 + off:(qc + 1) * 512],
                                start=True, stop=True)
                            info.append((kt, 512 * idx + off, off, N, rrel))
                        # exp over each contiguous written run (a diagonal
                        # non-first tile leaves an unwritten gap)
                        runs = []
                        for kt, base, off, N, rrel in info:
                            if runs and runs[-1][1] == base:
                                runs[-1][1] = base + N
                            else:
                                runs.append([base, base + N])
                        for lo, hi in runs:
                            nc.scalar.activation(out=ex[:, lo:hi],
                                                 in_=s_ps[:, lo:hi],
                                                 func=AF.Exp)
                        for kt, base, off, N, rrel in info:
                            if rrel >= 0:
                                nc.vector.tensor_mul(
                                    ex[:, base:base + 128],
                                    ex[:, base:base + 128], triu_sb[:])
                        pend[n] = (ex, info)

                    def emit_ypass(st, c0, cn):
                        y_ps = psC.tile([128, 512], F32, tag="y", name="y")
                        for cc in range(4):
                            w = 128 if cc < 3 else 64
                            lhsT = araw[cc][0:w, st * 128:(st + 1) * 128]
                            nc.tensor.matmul(y_ps[:, 0:cn], lhsT,
                                             wo_sb[0:w, cc, c0:c0 + cn],
                                             start=(cc == 0), stop=(cc == 3))
                        y_sb = tmp.tile([128, 512], F16, tag="ysb",
                                        name="ysb", bufs=4)
                        nc.vector.tensor_copy(y_sb[:, 0:cn], y_ps[:, 0:cn])
                        nc.sync.dma_start(
                            out=y[st * 128:(st + 1) * 128, c0:c0 + cn],
                            in_=y_sb[:, 0:cn])

                    pend_ypass = []

                    def flush_ypass(n, min_age=0):
                        for _ in range(n):
                            if not pend_ypass:
                                return
                            born, fn = pend_ypass[0]
                            if gctr[0] - born < min_age:
                                return
                            # CORRECTNESS: the qc's normalize multiplies must
                            # be emitted before its projection reads araw
                            if pend_muls and pend_muls[0][0] <= born:
                                return
                            pend_ypass.pop(0)
                            fn()

                    def unit_done(ui):
                        # evacuate o_ps + deferred-normalization hooks
                        h, qc = sched[ui]
                        half = (h % 2) * 64
                        o_ps = unit_ops.pop(ui)
                        nc.vector.tensor_copy(
                            araw[h // 2][half:half + 64,
                                         qc * 512:(qc + 1) * 512],
                            o_ps[0:64, :])
                        row = 32 * (h % 4)
                        nc.vector.tensor_copy(
                            den_t[h // 4][row:row + 1,
                                          qc * 512:(qc + 1) * 512],
                            o_ps[64:65, :])
                        if h == HL - 1:
                            # all 7 heads done with this qc: normalize both
                            # den tiles' chunks, then queue the qc's output
                            # projection (age-gated deeper than the muls)
                            qlo, qhi = qc * 512, (qc + 1) * 512
                            for j in (0, 1):
                                rec32 = tmp.tile([128, 512], F32,
                                                 tag="rec32c", name="rec32c",
                                                 bufs=2)
                                nc.vector.reciprocal_approx_fast(
                                    out=rec32[:], in_=den_t[j][:, qlo:qhi])
                                nc.vector.tensor_copy(rec16[j][:, qlo:qhi],
                                                      rec32[:])
                            for hh in range(HL):
                                hhalf = (hh % 2) * 64
                                jj, rw = hh // 4, 32 * (hh % 4)
                                nc.sync.dma_start(
                                    out=dram_rec[hh:hh + 1, qlo:qhi],
                                    in_=rec16[jj][rw:rw + 1, qlo:qhi])
                                rbF = tmp.tile([128, 512], rec_dt,
                                               tag="rbFc", name="rbFc",
                                               bufs=8)
                                nc.sync.dma_start(out=rbF[:], in_=bass.AP(
                                    tensor=dram_rec.tensor,
                                    offset=hh * s + qlo,
                                    ap=[[0, 128], [1, 512]]))

                                def _mul(hh=hh, hhalf=hhalf, qlo=qlo,
                                         qhi=qhi, rbF=rbF):
                                    nc.vector.tensor_mul(
                                        araw[hh // 2][hhalf:hhalf + 64,
                                                      qlo:qhi],
                                        araw[hh // 2][hhalf:hhalf + 64,
                                                      qlo:qhi],
                                        rbF[hhalf:hhalf + 64, :])
                                pend_muls.append((gctr[0], _mul))
                            if "C" in phases:
                                for st in range(4 * qc, 4 * qc + 4):
                                    for c0, cn in ((0, 512), (512, 384)):
                                        def _yp(st=st, c0=c0, cn=cn):
                                            emit_ypass(st, c0, cn)
                                        pend_ypass.append((gctr[0], _yp))

                    LOOK = 3
                    NG = len(gseq)
                    for n in range(min(LOOK, NG)):
                        emit_scores(n)
                    for n in range(NG):
                        # AV first, THEN the lookahead scores: the deeper
                        # lookahead's psum wait can't head-of-line block the
                        # ready AV group in the in-order PE stream
                        ui, kts = gseq[n]
                        h, qc = sched[ui]
                        nkt = 4 * (qc + 1)
                        if ui not in unit_ops:
                            unit_ops[ui] = psO.tile([65, 512], F32, tag="o",
                                                    name="o")
                        o_ps = unit_ops[ui]
                        ex, info = pend.pop(n)
                        for kt, base, off, N, rrel in info:
                            nc.tensor.matmul(
                                o_ps[:, off:512], v_all[:, kt, :],
                                ex[:, base:base + N],
                                start=(kt == 0), stop=(kt == nkt - 1))
                        if n + LOOK < NG:
                            emit_scores(n + LOOK)
                        gctr[0] += 1
                        flush_muls(3, min_age=6)
                        flush_ypass(1, min_age=10)
                        if kts[-1] == nkt - 1:
                            unit_done(ui)
                    flush_muls(len(pend_muls))
                    flush_ypass(len(pend_ypass))
                    if "C" not in phases or "B" not in phases:
                        y_sb = tmp.tile([128, 512], F16, tag="ysb", name="ysb",
                                        bufs=4)
                        nc.vector.memset(y_sb[:], 0.0)
                        nc.sync.dma_start(out=y[0:128, 0:512], in_=y_sb[:])

            if reps > 1:
                with tc.For_i(0, reps, 1):
                    _body()
            else:
                _body()

    nc.compile()
    return nc


# ---------------------------------------------------------------------------
# host-side sharding + execution
# ---------------------------------------------------------------------------

def round_f32r(a):
    """Round fp32 array to fp32r (RNE to 11 mantissa bits) -- bit-exact match
    of the hardware's casting DMA, verified by SBUF readback."""
    b = np.ascontiguousarray(a, dtype=np.float32).view(np.uint32)
    lsb = (b >> np.uint32(12)) & np.uint32(1)
    r = ((b + np.uint32(0x7FF) + lsb) & np.uint32(0xFFFFF000))
    return r.view(np.float32)


MM_DT = {"f32r": F32R, "f16": F16, "f32": F32}[os.environ.get("MM_DT", "f16")]


def _cvt(a, mm_dt):
    if mm_dt == F16:
        return np.ascontiguousarray(np.asarray(a, dtype=np.float32)).astype(np.float16)
    if mm_dt == F32R:
        return round_f32r(a)
    return np.ascontiguousarray(a, dtype=np.float32)


def make_in_maps(x, freqs_cos, freqs_sin, Wq, Wk, Wv, Wo, s=S, mm_dt=None):
    if mm_dt is None:
        mm_dt = MM_DT
    ST = s // 128
    scale = 1.0 / math.sqrt(D)
    cosr = np.ascontiguousarray(
        np.asarray(freqs_cos).reshape(ST, 128, 32).transpose(1, 0, 2)).astype(np.float32)
    sinr = np.ascontiguousarray(
        np.asarray(freqs_sin).reshape(ST, 128, 32).transpose(1, 0, 2)).astype(np.float32)
    triu = _cvt(np.triu(np.ones((128, 128), dtype=np.float32)), mm_dt)
    # per-head channel permutation [evens | odds] applied to q and k
    # (RoPE pairs become contiguous blocks on-device; scores are invariant)
    perm = np.concatenate([np.arange(0, D, 2), np.arange(1, D, 2)])
    qperm = np.concatenate([h * D + perm for h in range(HL)])
    in_maps = []
    for c in range(N_CORES):
        b, g = c // 2, c % 2
        in_maps.append({
            "xT": _cvt(np.asarray(x)[b].T, mm_dt),
            "wq": _cvt(np.concatenate(
                [np.asarray(Wq)[:, g * GD:(g + 1) * GD][:, qperm] * scale,
                 np.asarray(Wk)[:, g * D:(g + 1) * D][:, perm],
                 np.asarray(Wv)[:, g * D:(g + 1) * D]], axis=1), mm_dt),
            "wo": _cvt(np.asarray(Wo)[g * GD:(g + 1) * GD, :], mm_dt),
            "cosr": cosr, "sinr": sinr, "triu": triu,
        })
    return in_maps


_RUNNER = None


class _Runner:
    """Minimal SPMD executor over axon PJRT (self-contained copy)."""

    def __init__(self, nc, n_cores):
        import jax
        from jax.sharding import Mesh, PartitionSpec, NamedSharding
        from jax.experimental.shard_map import shard_map
        from concourse.bass2jax import (_bass_exec_p, install_neuronx_cc_hook,
                                        partition_id_tensor)
        install_neuronx_cc_hook()
        self.jax = jax
        self.n_cores = n_cores
        partition_name = (nc.partition_id_tensor.name
                          if nc.partition_id_tensor else None)
        in_names, out_names, out_avals = [], [], []
        for alloc in nc.m.functions[0].allocations:
            if not isinstance(alloc, mybir.MemoryLocationSet):
                continue
            name = alloc.memorylocations[0].name
            if alloc.kind == "ExternalInput":
                if name != partition_name:
                    in_names.append(name)
            elif alloc.kind == "ExternalOutput":
                out_names.append(name)
                out_avals.append(jax.core.ShapedArray(
                    tuple(alloc.tensor_shape), mybir.dt.np(alloc.dtype)))
        self.in_names, self.out_names, self.out_avals = in_names, out_names, out_avals
        n_params, n_outs = len(in_names), len(out_avals)
        all_names = in_names + out_names
        if partition_name is not None:
            all_names.append(partition_name)

        def _body(*args):
            operands = list(args)
            if partition_name is not None:
                operands.append(partition_id_tensor())
            return tuple(_bass_exec_p.bind(
                *operands, out_avals=tuple(out_avals), in_names=tuple(all_names),
                out_names=tuple(out_names), lowering_input_output_aliases=(),
                sim_require_finite=False, sim_require_nnan=False, nc=nc))

        devices = jax.devices()[:n_cores]
        self.mesh = Mesh(np.asarray(devices), ("core",))
        self.sharding = NamedSharding(self.mesh, PartitionSpec("core"))
        in_specs = (PartitionSpec("core"),) * (n_params + n_outs)
        out_specs = (PartitionSpec("core"),) * n_outs
        self.fn = jax.jit(
            shard_map(_body, mesh=self.mesh, in_specs=in_specs,
                      out_specs=out_specs, check_rep=False),
            donate_argnums=tuple(range(n_params, n_params + n_outs)),
            keep_unused=True)
        zshapes = [(n_cores * a.shape[0], *a.shape[1:]) for a in out_avals]
        zdtypes = [a.dtype for a in out_avals]
        self.make_zeros = jax.jit(
            lambda: tuple(jax.numpy.zeros(sh, dt)
                          for sh, dt in zip(zshapes, zdtypes)),
            out_shardings=tuple(self.sharding for _ in zshapes))

    def prep(self, in_maps):
        return [self.jax.device_put(
            np.concatenate([np.asarray(in_maps[c][n]) for c in range(self.n_cores)],
                           axis=0), self.sharding)
            for n in self.in_names]

    def run(self, dev_in):
        return self.fn(*dev_in, *self.make_zeros())

    def split(self, outs):
        res = []
        for c in range(self.n_cores):
            res.append({n: np.asarray(outs[i]).reshape(
                self.n_cores, *self.out_avals[i].shape)[c]
                for i, n in enumerate(self.out_names)})
        return res


def get_runner():
    global _RUNNER
    if _RUNNER is None:
        _RUNNER = _Runner(build(), N_CORES)
    return _RUNNER


def kernel(x, freqs_cos, freqs_sin, mask, Wq, Wk, Wv, Wo):
    x = np.asarray(x, dtype=np.float32)
    in_maps = make_in_maps(np.asarray(x), np.asarray(freqs_cos),
                           np.asarray(freqs_sin), np.asarray(Wq),
                           np.asarray(Wk), np.asarray(Wv), np.asarray(Wo))
    r = get_runner()
    outs = r.run(r.prep(in_maps))
    res = r.split(outs)
    out = np.empty((B, S, HID), dtype=np.float32)
    for b in range(B):
        out[b] = (res[2 * b]["y"].astype(np.float32)
                  + res[2 * b + 1]["y"].astype(np.float32))
    return out
